# revision 48
# baseline (speedup 1.0000x reference)
"""MoE layer (N=16384, D=1024, E=8, H=2048, top-2) on 8 trn2 NeuronCores.

Strategy: expert parallelism + routing-weight-aware mixed precision.
Core c owns expert c's weights; the host computes the gating
(bit-identically to the reference, CPU jax) and dispatches each token to
its two routed experts. Each expert's pairs are sorted by routing weight
p: the C_F lowest-p pairs run an fp8(e4m3) DoubleRow pipeline (2 fp8
MACs/PE/cycle), the C_B highest-p pairs a bf16 pipeline. Output error
scales with sum(p^2) over pairs, so spending fp8 on low-p pairs buys
most of the speed at a small accuracy cost. C_F + C_B = N*TOP_K/E =
capacity; fixed per-class counts keep all 8 cores' programs identical
and perfectly balanced. Overflow pairs (lowest p) are computed exactly
on the host.

fp8 scaling: W1 is quantized x32 and W2 x64 (power-of-two, folded back
out through the activation's input scale) so the weights sit in e4m3's
normal range; x and h fit e4m3 (|x| < 5 << 240) unscaled.

All device tensors are packed host-side into the exact SBUF tile
layouts so every DMA moves 2-8KB contiguous runs per partition.

Self-contained: only numpy/jax/ml_dtypes/concourse imports.
"""
import numpy as np

import concourse.bass as bass
import concourse.mybir as mybir
import concourse.tile as tile
from concourse.bass_utils import run_bass_kernel_spmd

N, D, E, H, TOP_K = 16384, 1024, 8, 2048, 2
P = 128
BMAIN = 512      # main token block (moving dim per matmul)
KD = D // P      # 8 k-tiles over D
JH = H // P      # 16 h-tiles over H
KH = KD // 2
NWARM = 24       # PE warmup matmuls issued while the startup DMAs stream

CAP = N * TOP_K // E   # 4096 pairs per expert on-device
C_F = 3328             # fp8-class pairs per expert (lowest routing weight)
C_B = CAP - C_F        # bf16-class pairs per expert (highest routing weight)
S1 = 32.0              # fp8 W1 pre-scale (power of two)
S2 = 64.0              # fp8 W2 pre-scale

TRACE = False          # test harness may flip this
TRACE_CORES = None     # e.g. list(range(8)) to profile every core
LAST_RESULTS = None    # BassKernelResults of the last device run

F32 = mybir.dt.float32
BF16 = mybir.dt.bfloat16
FP8 = mybir.dt.float8e4
DR = mybir.MatmulPerfMode.DoubleRow


def _split_excess_waits(nc, max_waits=1):
    """This walrus build rejects >1 sem-wait per instruction; Tile emits more.
    Move excess waits onto same-engine NOPs inserted right before."""
    for fn in nc.m.functions:
        for blk in fn.blocks:
            insts = list(blk.instructions)
            out = []
            changed = False
            for inst in insts:
                si = getattr(inst, "sync_info", None)
                if si is not None and si.on_wait and len(si.on_wait) > max_waits:
                    waits = list(si.on_wait)
                    excess, keep = waits[:-max_waits], waits[-max_waits:]
                    for i in range(0, len(excess), max_waits):
                        out.append(
                            mybir.InstNoOp(
                                name=nc.get_next_instruction_name(),
                                engine=inst.engine,
                                sync_info=mybir.SyncInfo(
                                    on_wait=excess[i : i + max_waits], on_update=[]
                                ),
                                bass_nofuse=True,
                            )
                        )
                    inst.sync_info = mybir.SyncInfo(
                        on_wait=keep, on_update=list(si.on_update)
                    )
                    changed = True
                out.append(inst)
            if changed:
                blk.instructions = out


def _plan_blocks(C):
    """512-wide blocks; a sub-512 remainder becomes one block in [256,512] or
    two (rem-256, 256) blocks so every matmul keeps an efficient moving dim.
    Tail blocks go last."""
    blocks, off = [], 0
    while C - off >= BMAIN + 256:
        blocks.append((off, BMAIN))
        off += BMAIN
    rem = C - off
    if rem > BMAIN:
        blocks.append((off, rem - 256))
        blocks.append((off + rem - 256, 256))
    elif rem:
        blocks.append((off, rem))
    return blocks


def _plan_blocks_f(cf):
    """fp8 phase plan. Block 0's x arrives as one full-tile DMA on the
    fast gpsimd ring, so a full 512 first block costs little startup
    latency and buys ~8us of weight-chunk deadline slack."""
    return _plan_blocks(cf)


def build_nc(cb: int, cf: int):
    """Per-core dense expert MLP y = gelu(x @ w1 + b1) @ w2 + b2 over two
    token classes: cf fp8 DoubleRow tokens then cb bf16 tokens."""
    nc = bass.Bass("TRN2", target_bir_lowering=False)
    havef, haveb = cf > 0, cb > 0
    if havef:
        xpkf = nc.dram_tensor("xpkf", (P, KD * cf), FP8, kind="ExternalInput")
        w1pkf = nc.dram_tensor("w1pkf", (P, JH * KD * P), FP8, kind="ExternalInput")
        w2pkf = nc.dram_tensor("w2pkf", (P, KD * JH * P), FP8, kind="ExternalInput")
    if haveb:
        xpk = nc.dram_tensor("xpk", (P, KD * cb), BF16, kind="ExternalInput")
        w1pk = nc.dram_tensor("w1pk", (P, JH * KD * P), BF16, kind="ExternalInput")
        w2pk = nc.dram_tensor("w2pk", (P, KD * JH * P), BF16, kind="ExternalInput")
    b1v = nc.dram_tensor("b1v", (P, JH), F32, kind="ExternalInput")
    b2v = nc.dram_tensor("b2v", (P, KD), F32, kind="ExternalInput")
    ypk = nc.dram_tensor("ypk", (P, KD * (cb + cf)), BF16, kind="ExternalOutput")

    with tile.TileContext(nc) as tc:
        with (
            tc.tile_pool(name="wpool", bufs=1) as wpool,
            tc.tile_pool(name="xpool", bufs=2) as xpool,
            tc.tile_pool(name="hpool", bufs=2) as hpool,
            tc.tile_pool(name="ypool", bufs=2) as ypool,
            tc.tile_pool(name="psum", bufs=3, space="PSUM") as psum,
            tc.tile_pool(name="wpsum", bufs=1, space="PSUM") as wpsum,
        ):
            blocks_f = _plan_blocks_f(cf) if havef else []
            blocks_b = _plan_blocks(cb) if haveb else []

            # PE warmup: matmuls on a memset tile with no DMA dependencies.
            # They run during the startup DMA burst so the HAM clock gate is
            # already at 8/8 when the first real matmul issues.
            # memset on the (otherwise idle) vector engine: gpsimd's first
            # instruction must be the block-0 x DMA trigger, not this
            wzero = wpool.tile([P, 256], BF16, name="wzero")
            nc.vector.memset(wzero[:], 0.0)
            pwarm = wpsum.tile([P, 256], F32, tag="pwarm")

            def warm(n):
                for _ in range(n):
                    nc.tensor.matmul(
                        pwarm[:], wzero[:, :P], wzero[:], start=True, stop=True
                    )

            warm(NWARM)

            def load_block(off, B, eng, xsrc, dt, tagpfx):
                # two half-tiles (separate tiles, one DMA each): a single
                # full-tile DMA per mid-stream block measured +6us — the
                # half split keeps the x pipeline finer-grained
                xa = xpool.tile([P, KH, B], dt, tag=f"{tagpfx}a")
                eng.dma_start(xa[:], xsrc[:, KD * off : KD * off + KH * B])
                xc = xpool.tile([P, KH, B], dt, tag=f"{tagpfx}c")
                eng.dma_start(xc[:], xsrc[:, KD * off + KH * B : KD * (off + B)])
                return xa, xc

            # Weights live in chunked tiles (a few j/d planes per tile) so
            # every weight DMA moves 2-8KB contiguous per partition: 1KB
            # runs are descriptor-bound at ~3GB/s per ring, 2KB+ runs reach
            # ~14GB/s per ring engine.
            def make_chunks(chunks, shape_of, dt, pfx):
                tiles = []
                for (o, n) in chunks:
                    tiles.append(wpool.tile(shape_of(n), dt, name=f"{pfx}{o}"))
                def at(i):
                    for (o, n), t in zip(chunks, tiles):
                        if o <= i < o + n:
                            return t, i - o
                    raise IndexError(i)
                return tiles, at

            # 3D chunk tiles [P, n*KD, P] keep the matmul weight APs in the
            # exact [P, pair, 128] / [P, 128] shapes the lowering expects;
            # the plane index folds into the middle-dim offset.
            if havef:
                w1f_chunks = [(0, 2), (2, 2), (4, 4), (8, 8)]
                w1f_tiles, w1f_at = make_chunks(
                    w1f_chunks, lambda n: [P, n * KD, P], FP8, "w1f")
                w2f_chunks = [(0, 4), (4, 4)]
                w2f_tiles, w2f_at = make_chunks(
                    w2f_chunks, lambda n: [P, n * JH, P], FP8, "w2f")
            if haveb:
                w1b_chunks = [(0, 4), (4, 4), (8, 4), (12, 4)]
                w1b_tiles, w1b_at = make_chunks(
                    w1b_chunks, lambda n: [P, n * KD, P], BF16, "w1b")
                w2b_chunks = [(0, 2), (2, 2), (4, 2), (6, 2)]
                w2b_tiles, w2b_at = make_chunks(
                    w2b_chunks, lambda n: [P, n * JH, P], BF16, "w2b")

            b1sb = wpool.tile([P, JH], F32)
            b2sb = wpool.tile([P, KD], F32)

            def load_w1f(ci, eng):
                o, n = w1f_chunks[ci]
                eng.dma_start(w1f_tiles[ci][:], w1pkf[:, o * KD * P : (o + n) * KD * P])

            def load_w2f(ci, eng):
                o, n = w2f_chunks[ci]
                eng.dma_start(w2f_tiles[ci][:], w2pkf[:, o * JH * P : (o + n) * JH * P])

            def load_w1b(ci, eng):
                o, n = w1b_chunks[ci]
                eng.dma_start(w1b_tiles[ci][:], w1pk[:, o * KD * P : (o + n) * KD * P])

            def load_w2b(ci, eng):
                o, n = w2b_chunks[ci]
                eng.dma_start(w2b_tiles[ci][:], w2pk[:, o * JH * P : (o + n) * JH * P])

            # Startup: the first phase's weight set + first x block spread
            # over all three rings in deadline order; the bulk (second
            # phase's weights) follows on gpsimd behind a short delay.
            if havef:
                # The gpsimd SWDGE ring moves 2KB+/partition runs at
                # ~200GB/s while the sync/scalar rings crawl at ~50GB/s on
                # startup, so the ENTIRE first-block critical set (block-0
                # x as one full-tile DMA, then every fp8 weight chunk in
                # deadline order) rides gpsimd; sync prefetches the later
                # x blocks; scalar only carries biases + y writebacks.
                B0 = blocks_f[0][1]
                # block-0 x split over TWO rings in parallel (separate
                # tiles — two DMAs into one tile is the serialization
                # pitfall): gpsimd carries k0-5 + weights, sync k6-7
                xf0 = xpool.tile([P, KD - 2, B0], FP8, tag="xff")
                nc.gpsimd.dma_start(xf0[:], xpkf[:, : (KD - 2) * B0])
                xf0b = xpool.tile([P, 2, B0], FP8, tag="xffb")
                nc.sync.dma_start(xf0b[:], xpkf[:, (KD - 2) * B0 : KD * B0])
                for ci in range(len(w1f_chunks)):
                    load_w1f(ci, nc.gpsimd)
                for ci in range(len(w2f_chunks)):
                    load_w2f(ci, nc.gpsimd)
                nc.scalar.dma_start(b1sb[:], b1v[:])
                nc.scalar.dma_start(b2sb[:], b2v[:])
                dly2 = wpool.tile([P, 512], F32, name="dly2")
                for _ in range(4):
                    nc.gpsimd.memset(dly2[:], 0.0)
                if haveb:
                    for ci in range(len(w1b_chunks)):
                        load_w1b(ci, nc.gpsimd)
                    for ci in range(len(w2b_chunks)):
                        load_w2b(ci, nc.gpsimd)
            else:
                xb0 = load_block(*blocks_b[0], nc.sync, xpk, BF16, "xb_")
                load_w1b(0, nc.sync)
                load_w1b(1, nc.sync)
                nc.scalar.dma_start(b1sb[:], b1v[:])
                load_w1b(2, nc.scalar)
                nc.scalar.dma_start(b2sb[:], b2v[:])
                dly2 = wpool.tile([P, 512], F32, name="dly2")
                for _ in range(4):
                    nc.gpsimd.memset(dly2[:], 0.0)
                load_w1b(3, nc.gpsimd)
                for ci in range(len(w2b_chunks)):
                    load_w2b(ci, nc.gpsimd)

            nblk = len(blocks_f) + len(blocks_b)
            bi = 0

            # ── fp8 DoubleRow phase ──────────────────────────────────────
            for fi, (off, B) in enumerate(blocks_f):
                if fi == 0:
                    xa = xc = None
                else:
                    xa, xc = load_block(off, B, nc.sync, xpkf, FP8, "xf_")
                hb = hpool.tile([P, JH, B], FP8, tag="hbf")
                for j in range(JH):
                    w1t, jj = w1f_at(j)
                    ph = psum.tile([P, B], F32, tag="ph")
                    for kp in range(KD // 2):
                        if fi == 0:
                            src = (
                                xf0[:, 2 * kp : 2 * kp + 2]
                                if kp < KD // 2 - 1
                                else xf0b[:, 0:2]
                            )
                        elif kp < KH // 2:
                            src = xa[:, 2 * kp : 2 * kp + 2]
                        else:
                            src = xc[:, 2 * (kp - KH // 2) : 2 * (kp - KH // 2) + 2]
                        nc.tensor.matmul(
                            ph[:],
                            w1t[:, jj * KD + 2 * kp : jj * KD + 2 * kp + 2],
                            src,
                            start=(kp == 0),
                            stop=(kp == KD // 2 - 1),
                            perf_mode=DR,
                        )
                    nc.scalar.activation(
                        hb[:, j],
                        ph[:],
                        mybir.ActivationFunctionType.Gelu,
                        bias=b1sb[:, j : j + 1],
                        scale=1.0 / S1,
                    )
                    if bi == 0 and j < 4:
                        # gap-fillers: keep the PE busy through the early
                        # DMA waits so the HAM clock gate never re-throttles
                        warm(4)
                yst = ypool.tile([P, KD, B], BF16, tag="yst")
                last = bi == nblk - 1
                for d in range(KD):
                    w2t, dd = w2f_at(d)
                    pd = psum.tile([P, B], F32, tag="pd")
                    for jp in range(JH // 2):
                        nc.tensor.matmul(
                            pd[:],
                            w2t[:, dd * JH + 2 * jp : dd * JH + 2 * jp + 2],
                            hb[:, 2 * jp : 2 * jp + 2],
                            start=(jp == 0),
                            stop=(jp == JH // 2 - 1),
                            perf_mode=DR,
                        )
                    nc.scalar.activation(
                        yst[:, d],
                        pd[:],
                        mybir.ActivationFunctionType.Identity,
                        bias=b2sb[:, d : d + 1],
                        scale=1.0 / S2,
                    )
                    if last and d % 2 == 1:
                        # drain the finished pair right away so the
                        # end-of-kernel barrier waits on 2 planes, not 8
                        nc.sync.dma_start(
                            ypk[:, KD * off + (d - 1) * B : KD * off + (d + 1) * B],
                            yst[:, d - 1 : d + 1],
                        )
                if not last:
                    nc.sync.dma_start(ypk[:, KD * off : KD * (off + B)], yst[:])
                bi += 1

            # ── bf16 phase ───────────────────────────────────────────────
            yoff = cf
            for bj, (off, B) in enumerate(blocks_b):
                if bj == 0 and not havef:
                    xa, xc = xb0
                else:
                    xa, xc = load_block(off, B, nc.sync, xpk, BF16, "xb_")
                hb = hpool.tile([P, JH, B], BF16, tag="hb")
                for j in range(JH):
                    w1t, jj = w1b_at(j)
                    ph = psum.tile([P, B], F32, tag="ph")
                    for k in range(KD):
                        nc.tensor.matmul(
                            ph[:],
                            w1t[:, jj * KD + k],
                            xa[:, k] if k < KH else xc[:, k - KH],
                            start=(k == 0),
                            stop=(k == KD - 1),
                        )
                    nc.scalar.activation(
                        hb[:, j],
                        ph[:],
                        mybir.ActivationFunctionType.Gelu,
                        bias=b1sb[:, j : j + 1],
                    )
                    if bi == 0 and j < 4:
                        warm(4)
                yst = ypool.tile([P, KD, B], BF16, tag="yst")
                last = bi == nblk - 1
                oo = yoff + off
                for d in range(KD):
                    w2t, dd = w2b_at(d)
                    pd = psum.tile([P, B], F32, tag="pd")
                    for j in range(JH):
                        nc.tensor.matmul(
                            pd[:],
                            w2t[:, dd * JH + j],
                            hb[:, j],
                            start=(j == 0),
                            stop=(j == JH - 1),
                        )
                    nc.scalar.activation(
                        yst[:, d],
                        pd[:],
                        mybir.ActivationFunctionType.Identity,
                        bias=b2sb[:, d : d + 1],
                    )
                    if last and d % 2 == 1:
                        nc.sync.dma_start(
                            ypk[:, KD * oo + (d - 1) * B : KD * oo + (d + 1) * B],
                            yst[:, d - 1 : d + 1],
                        )
                if not last:
                    nc.sync.dma_start(ypk[:, KD * oo : KD * (oo + B)], yst[:])
                bi += 1
    _split_excess_waits(nc)
    return nc


_NC_CACHE = {}


def _routing(x, Wg, bg):
    """Gating computed the same way (and on the same platform: CPU jax) as the
    reference, so the top-2 choice is bit-identical even for near-tie logits."""
    import jax
    import jax.numpy as jnp

    cpu = jax.local_devices(backend="cpu")[0]
    with jax.default_device(cpu):
        logits = jnp.asarray(x) @ jnp.asarray(Wg) + jnp.asarray(bg)
        probs = jax.nn.softmax(logits, axis=-1)
        topk_p, topk_i = jax.lax.top_k(probs, TOP_K)
        topk_p = topk_p / topk_p.sum(axis=-1, keepdims=True)
    return np.asarray(topk_i), np.asarray(topk_p)


def _pack_x(xg, C, blocks, dt):
    """xg (C, D) -> (P, KD*C): per block, k-major then token-major, so each
    xa/xc DMA reads one contiguous run per partition."""
    x3 = np.asarray(xg, dtype=dt).reshape(C, KD, P)
    parts = [
        np.transpose(x3[off : off + B], (2, 1, 0)).reshape(P, KD * B)
        for off, B in blocks
    ]
    return np.ascontiguousarray(np.concatenate(parts, axis=1))


def _unpack_y(ypk, C, blocks):
    """(P, KD*C) bf16 -> (C, D) fp32, inverse of the yst tile layout."""
    y = np.empty((C, D), np.float32)
    for off, B in blocks:
        blk = ypk[:, KD * off : KD * (off + B)].reshape(P, KD, B)
        y[off : off + B] = np.transpose(blk, (2, 1, 0)).reshape(B, D)
    return y


def _pack_w1(w, dt):
    # w1pk[p, j, k, q] = w[k*P+p, j*P+q]
    return np.ascontiguousarray(
        np.transpose(np.asarray(w, dtype=dt).reshape(KD, P, JH, P), (1, 2, 0, 3)
                     ).reshape(P, JH * KD * P))


def _pack_w2(w, dt):
    # w2pk[p, d, j, q] = w[j*P+p, d*P+q]
    return np.ascontiguousarray(
        np.transpose(np.asarray(w, dtype=dt).reshape(JH, P, KD, P), (1, 2, 0, 3)
                     ).reshape(P, KD * JH * P))


def kernel(x, Wg, bg, W1, b1, W2, b2):
    global LAST_RESULTS
    import ml_dtypes

    bf16 = ml_dtypes.bfloat16
    fp8 = ml_dtypes.float8_e4m3
    x = np.ascontiguousarray(np.asarray(x, dtype=np.float32))
    Wg = np.asarray(Wg, dtype=np.float32)
    bg = np.asarray(bg, dtype=np.float32)
    W1 = np.asarray(W1, dtype=np.float32)
    b1 = np.asarray(b1, dtype=np.float32)
    W2 = np.asarray(W2, dtype=np.float32)
    b2 = np.asarray(b2, dtype=np.float32)

    topk_i, topk_p = _routing(x, Wg, bg)

    # Per expert: pairs sorted by routing weight ascending. Lowest-p pairs
    # beyond capacity spill to the exact host path; of the on-device pairs
    # the top C_B by p go to the bf16 class, the rest to the fp8 class.
    idx_list, p_list = [], []
    overflow = []
    for e in range(E):
        m0 = topk_i[:, 0] == e
        m1 = topk_i[:, 1] == e
        idx = np.nonzero(m0 | m1)[0]
        p = np.where(m0[idx], topk_p[idx, 0], topk_p[idx, 1]).astype(np.float32)
        order = np.argsort(p, kind="stable")
        idx, p = idx[order], p[order]
        n = len(idx)
        if n > CAP:
            overflow.append((e, idx[: n - CAP], p[: n - CAP]))
            idx, p = idx[n - CAP :], p[n - CAP :]
        idx_list.append(idx)
        p_list.append(p)

    blocks_f = _plan_blocks_f(C_F) if C_F else []
    blocks_b = _plan_blocks(C_B) if C_B else []

    key = (C_B, C_F)
    if key not in _NC_CACHE:
        _NC_CACHE[key] = build_nc(C_B, C_F)
    nc = _NC_CACHE[key]

    in_maps = []
    nf_list, nb_list = [], []
    for e in range(E):
        idx = idx_list[e]
        n = len(idx)
        nb = min(C_B, n)            # top-p pairs -> bf16 class
        nf = min(C_F, n - nb)       # rest -> fp8 class
        nf_list.append(nf)
        nb_list.append(nb)
        im = {
            "b1v": np.ascontiguousarray(b1[e].reshape(JH, P).T),
            "b2v": np.ascontiguousarray(b2[e].reshape(KD, P).T),
        }
        if C_F:
            xgf = np.zeros((C_F, D), np.float32)
            xgf[:nf] = x[idx[:nf]]
            im["xpkf"] = _pack_x(xgf, C_F, blocks_f, fp8)
            im["w1pkf"] = _pack_w1(W1[e] * S1, fp8)
            im["w2pkf"] = _pack_w2(W2[e] * S2, fp8)
        if C_B:
            xgb = np.zeros((C_B, D), np.float32)
            xgb[:nb] = x[idx[nf : nf + nb]]
            im["xpk"] = _pack_x(xgb, C_B, blocks_b, bf16)
            im["w1pk"] = _pack_w1(W1[e], bf16)
            im["w2pk"] = _pack_w2(W2[e], bf16)
        in_maps.append(im)

    res = run_bass_kernel_spmd(
        nc, in_maps, core_ids=list(range(E)), trace=TRACE, trace_cores=TRACE_CORES
    )
    LAST_RESULTS = res

    out = x.copy()
    for e in range(E):
        idx, p = idx_list[e], p_list[e]
        nf, nb = nf_list[e], nb_list[e]
        ype = np.asarray(res.results[e]["ypk"], np.float32)
        yf = _unpack_y(ype[:, : KD * C_F], C_F, blocks_f) if C_F else None
        yb = _unpack_y(ype[:, KD * C_F :], C_B, blocks_b) if C_B else None
        if nf:
            out[idx[:nf]] += yf[:nf] * p[:nf, None]
        if nb:
            out[idx[nf : nf + nb]] += yb[:nb] * p[nf : nf + nb, None]
    if overflow:
        import jax
        import jax.numpy as jnp

        cpu = jax.local_devices(backend="cpu")[0]
        with jax.default_device(cpu):
            for e, didx, dp in overflow:
                h = jax.nn.gelu(
                    jnp.asarray(x[didx]) @ jnp.asarray(W1[e]) + b1[e],
                    approximate=False,
                )
                ye = np.asarray(h @ jnp.asarray(W2[e]) + b2[e])
                out[didx] += ye * dp[:, None]
    return out


# revision 50
# speedup vs baseline: 1.0055x; 1.0055x over previous
"""MoE layer (N=16384, D=1024, E=8, H=2048, top-2) on 8 trn2 NeuronCores.

Strategy: expert parallelism + routing-weight-aware mixed precision.
Core c owns expert c's weights; the host computes the gating
(bit-identically to the reference, CPU jax) and dispatches each token to
its two routed experts. Each expert's pairs are sorted by routing weight
p: the C_F lowest-p pairs run an fp8(e4m3) DoubleRow pipeline (2 fp8
MACs/PE/cycle), the C_B highest-p pairs a bf16 pipeline. Output error
scales with sum(p^2) over pairs, so spending fp8 on low-p pairs buys
most of the speed at a small accuracy cost. C_F + C_B = N*TOP_K/E =
capacity; fixed per-class counts keep all 8 cores' programs identical
and perfectly balanced. Overflow pairs (lowest p) are computed exactly
on the host.

fp8 scaling: W1 is quantized x32 and W2 x64 (power-of-two, folded back
out through the activation's input scale) so the weights sit in e4m3's
normal range; x and h fit e4m3 (|x| < 5 << 240) unscaled.

All device tensors are packed host-side into the exact SBUF tile
layouts so every DMA moves 2-8KB contiguous runs per partition.

Self-contained: only numpy/jax/ml_dtypes/concourse imports.
"""
import numpy as np

import concourse.bass as bass
import concourse.mybir as mybir
import concourse.tile as tile
from concourse.bass_utils import run_bass_kernel_spmd

N, D, E, H, TOP_K = 16384, 1024, 8, 2048, 2
P = 128
BMAIN = 512      # main token block (moving dim per matmul)
KD = D // P      # 8 k-tiles over D
JH = H // P      # 16 h-tiles over H
KH = KD // 2
NWARM = 24       # PE warmup matmuls issued while the startup DMAs stream

CAP = N * TOP_K // E   # 4096 pairs per expert on-device
C_F = 3328             # fp8-class pairs per expert (lowest routing weight)
C_B = CAP - C_F        # bf16-class pairs per expert (highest routing weight)
S1 = 32.0              # fp8 W1 pre-scale (power of two)
S2 = 64.0              # fp8 W2 pre-scale

TRACE = False          # test harness may flip this
TRACE_CORES = None     # e.g. list(range(8)) to profile every core
LAST_RESULTS = None    # BassKernelResults of the last device run

F32 = mybir.dt.float32
BF16 = mybir.dt.bfloat16
FP8 = mybir.dt.float8e4
DR = mybir.MatmulPerfMode.DoubleRow


def _split_excess_waits(nc, max_waits=1):
    """This walrus build rejects >1 sem-wait per instruction; Tile emits more.
    Move excess waits onto same-engine NOPs inserted right before."""
    for fn in nc.m.functions:
        for blk in fn.blocks:
            insts = list(blk.instructions)
            out = []
            changed = False
            for inst in insts:
                si = getattr(inst, "sync_info", None)
                if si is not None and si.on_wait and len(si.on_wait) > max_waits:
                    waits = list(si.on_wait)
                    excess, keep = waits[:-max_waits], waits[-max_waits:]
                    for i in range(0, len(excess), max_waits):
                        out.append(
                            mybir.InstNoOp(
                                name=nc.get_next_instruction_name(),
                                engine=inst.engine,
                                sync_info=mybir.SyncInfo(
                                    on_wait=excess[i : i + max_waits], on_update=[]
                                ),
                                bass_nofuse=True,
                            )
                        )
                    inst.sync_info = mybir.SyncInfo(
                        on_wait=keep, on_update=list(si.on_update)
                    )
                    changed = True
                out.append(inst)
            if changed:
                blk.instructions = out


def _plan_blocks(C):
    """512-wide blocks; a sub-512 remainder becomes one block in [256,512] or
    two (rem-256, 256) blocks so every matmul keeps an efficient moving dim.
    Tail blocks go last."""
    blocks, off = [], 0
    while C - off >= BMAIN + 256:
        blocks.append((off, BMAIN))
        off += BMAIN
    rem = C - off
    if rem > BMAIN:
        blocks.append((off, rem - 256))
        blocks.append((off + rem - 256, 256))
    elif rem:
        blocks.append((off, rem))
    return blocks


def _plan_blocks_f(cf):
    """fp8 phase plan. Block 0's x arrives as one full-tile DMA on the
    fast gpsimd ring, so a full 512 first block costs little startup
    latency and buys ~8us of weight-chunk deadline slack."""
    return _plan_blocks(cf)


def build_nc(cb: int, cf: int):
    """Per-core dense expert MLP y = gelu(x @ w1 + b1) @ w2 + b2 over two
    token classes: cf fp8 DoubleRow tokens then cb bf16 tokens."""
    nc = bass.Bass("TRN2", target_bir_lowering=False)
    havef, haveb = cf > 0, cb > 0
    if havef:
        xpkf = nc.dram_tensor("xpkf", (P, KD * cf), FP8, kind="ExternalInput")
        w1pkf = nc.dram_tensor("w1pkf", (P, JH * KD * P), FP8, kind="ExternalInput")
        w2pkf = nc.dram_tensor("w2pkf", (P, KD * JH * P), FP8, kind="ExternalInput")
    if haveb:
        xpk = nc.dram_tensor("xpk", (P, KD * cb), BF16, kind="ExternalInput")
        w1pk = nc.dram_tensor("w1pk", (P, JH * KD * P), BF16, kind="ExternalInput")
        w2pk = nc.dram_tensor("w2pk", (P, KD * JH * P), BF16, kind="ExternalInput")
    b1v = nc.dram_tensor("b1v", (P, JH), F32, kind="ExternalInput")
    b2v = nc.dram_tensor("b2v", (P, KD), F32, kind="ExternalInput")
    ypk = nc.dram_tensor("ypk", (P, KD * (cb + cf)), BF16, kind="ExternalOutput")

    with tile.TileContext(nc) as tc:
        with (
            tc.tile_pool(name="wpool", bufs=1) as wpool,
            tc.tile_pool(name="xpool", bufs=2) as xpool,
            tc.tile_pool(name="hpool", bufs=2) as hpool,
            tc.tile_pool(name="ypool", bufs=2) as ypool,
            tc.tile_pool(name="psum", bufs=3, space="PSUM") as psum,
            tc.tile_pool(name="wpsum", bufs=1, space="PSUM") as wpsum,
        ):
            blocks_f = _plan_blocks_f(cf) if havef else []
            blocks_b = _plan_blocks(cb) if haveb else []

            # PE warmup: matmuls on a memset tile with no DMA dependencies.
            # They run during the startup DMA burst so the HAM clock gate is
            # already at 8/8 when the first real matmul issues.
            # memset on the (otherwise idle) vector engine: gpsimd's first
            # instruction must be the block-0 x DMA trigger, not this
            wzero = wpool.tile([P, 256], BF16, name="wzero")
            nc.vector.memset(wzero[:], 0.0)
            pwarm = wpsum.tile([P, 256], F32, tag="pwarm")

            def warm(n):
                for _ in range(n):
                    nc.tensor.matmul(
                        pwarm[:], wzero[:, :P], wzero[:], start=True, stop=True
                    )

            warm(NWARM)

            def load_block(off, B, eng, xsrc, dt, tagpfx):
                # two half-tiles (separate tiles, one DMA each): a single
                # full-tile DMA per mid-stream block measured +6us — the
                # half split keeps the x pipeline finer-grained
                xa = xpool.tile([P, KH, B], dt, tag=f"{tagpfx}a")
                eng.dma_start(xa[:], xsrc[:, KD * off : KD * off + KH * B])
                xc = xpool.tile([P, KH, B], dt, tag=f"{tagpfx}c")
                eng.dma_start(xc[:], xsrc[:, KD * off + KH * B : KD * (off + B)])
                return xa, xc

            # Weights live in chunked tiles (a few j/d planes per tile) so
            # every weight DMA moves 2-8KB contiguous per partition: 1KB
            # runs are descriptor-bound at ~3GB/s per ring, 2KB+ runs reach
            # ~14GB/s per ring engine.
            def make_chunks(chunks, shape_of, dt, pfx):
                tiles = []
                for (o, n) in chunks:
                    tiles.append(wpool.tile(shape_of(n), dt, name=f"{pfx}{o}"))
                def at(i):
                    for (o, n), t in zip(chunks, tiles):
                        if o <= i < o + n:
                            return t, i - o
                    raise IndexError(i)
                return tiles, at

            # 3D chunk tiles [P, n*KD, P] keep the matmul weight APs in the
            # exact [P, pair, 128] / [P, 128] shapes the lowering expects;
            # the plane index folds into the middle-dim offset.
            if havef:
                w1f_chunks = [(0, 2), (2, 2), (4, 4), (8, 8)]
                w1f_tiles, w1f_at = make_chunks(
                    w1f_chunks, lambda n: [P, n * KD, P], FP8, "w1f")
                w2f_chunks = [(0, 4), (4, 4)]
                w2f_tiles, w2f_at = make_chunks(
                    w2f_chunks, lambda n: [P, n * JH, P], FP8, "w2f")
            if haveb:
                w1b_chunks = [(0, 4), (4, 4), (8, 4), (12, 4)]
                w1b_tiles, w1b_at = make_chunks(
                    w1b_chunks, lambda n: [P, n * KD, P], BF16, "w1b")
                w2b_chunks = [(0, 2), (2, 2), (4, 2), (6, 2)]
                w2b_tiles, w2b_at = make_chunks(
                    w2b_chunks, lambda n: [P, n * JH, P], BF16, "w2b")

            b1sb = wpool.tile([P, JH], F32)
            b2sb = wpool.tile([P, KD], F32)

            def load_w1f(ci, eng):
                o, n = w1f_chunks[ci]
                eng.dma_start(w1f_tiles[ci][:], w1pkf[:, o * KD * P : (o + n) * KD * P])

            def load_w2f(ci, eng):
                o, n = w2f_chunks[ci]
                eng.dma_start(w2f_tiles[ci][:], w2pkf[:, o * JH * P : (o + n) * JH * P])

            def load_w1b(ci, eng):
                o, n = w1b_chunks[ci]
                eng.dma_start(w1b_tiles[ci][:], w1pk[:, o * KD * P : (o + n) * KD * P])

            def load_w2b(ci, eng):
                o, n = w2b_chunks[ci]
                eng.dma_start(w2b_tiles[ci][:], w2pk[:, o * JH * P : (o + n) * JH * P])

            # Startup: the first phase's weight set + first x block spread
            # over all three rings in deadline order; the bulk (second
            # phase's weights) follows on gpsimd behind a short delay.
            if havef:
                # The gpsimd SWDGE ring moves 2KB+/partition runs at
                # ~200GB/s while the sync/scalar rings crawl at ~50GB/s on
                # startup, so the ENTIRE first-block critical set (block-0
                # x as one full-tile DMA, then every fp8 weight chunk in
                # deadline order) rides gpsimd; sync prefetches the later
                # x blocks; scalar only carries biases + y writebacks.
                B0 = blocks_f[0][1]
                xf0 = xpool.tile([P, KD, B0], FP8, tag="xff")
                # one full-tile DMA on gpsimd; splitting k6-7 onto sync in
                # parallel measured neutral (gap 2.8 vs 2.5us, PE +0.8us)
                nc.gpsimd.dma_start(xf0[:], xpkf[:, : KD * B0])
                for ci in range(len(w1f_chunks)):
                    load_w1f(ci, nc.gpsimd)
                for ci in range(len(w2f_chunks)):
                    load_w2f(ci, nc.gpsimd)
                nc.scalar.dma_start(b1sb[:], b1v[:])
                nc.scalar.dma_start(b2sb[:], b2v[:])
                dly2 = wpool.tile([P, 512], F32, name="dly2")
                for _ in range(4):
                    nc.gpsimd.memset(dly2[:], 0.0)
                if haveb:
                    for ci in range(len(w1b_chunks)):
                        load_w1b(ci, nc.gpsimd)
                    for ci in range(len(w2b_chunks)):
                        load_w2b(ci, nc.gpsimd)
            else:
                xb0 = load_block(*blocks_b[0], nc.sync, xpk, BF16, "xb_")
                load_w1b(0, nc.sync)
                load_w1b(1, nc.sync)
                nc.scalar.dma_start(b1sb[:], b1v[:])
                load_w1b(2, nc.scalar)
                nc.scalar.dma_start(b2sb[:], b2v[:])
                dly2 = wpool.tile([P, 512], F32, name="dly2")
                for _ in range(4):
                    nc.gpsimd.memset(dly2[:], 0.0)
                load_w1b(3, nc.gpsimd)
                for ci in range(len(w2b_chunks)):
                    load_w2b(ci, nc.gpsimd)

            nblk = len(blocks_f) + len(blocks_b)
            bi = 0

            # ── fp8 DoubleRow phase ──────────────────────────────────────
            for fi, (off, B) in enumerate(blocks_f):
                if fi == 0:
                    xa = xc = None
                else:
                    xa, xc = load_block(off, B, nc.sync, xpkf, FP8, "xf_")
                hb = hpool.tile([P, JH, B], FP8, tag="hbf")
                for j in range(JH):
                    w1t, jj = w1f_at(j)
                    ph = psum.tile([P, B], F32, tag="ph")
                    for kp in range(KD // 2):
                        if fi == 0:
                            src = xf0[:, 2 * kp : 2 * kp + 2]
                        elif kp < KH // 2:
                            src = xa[:, 2 * kp : 2 * kp + 2]
                        else:
                            src = xc[:, 2 * (kp - KH // 2) : 2 * (kp - KH // 2) + 2]
                        nc.tensor.matmul(
                            ph[:],
                            w1t[:, jj * KD + 2 * kp : jj * KD + 2 * kp + 2],
                            src,
                            start=(kp == 0),
                            stop=(kp == KD // 2 - 1),
                            perf_mode=DR,
                        )
                    nc.scalar.activation(
                        hb[:, j],
                        ph[:],
                        mybir.ActivationFunctionType.Gelu,
                        bias=b1sb[:, j : j + 1],
                        scale=1.0 / S1,
                    )
                    if bi == 0 and j < 4:
                        # gap-fillers: keep the PE busy through the early
                        # DMA waits so the HAM clock gate never re-throttles
                        warm(4)
                yst = ypool.tile([P, KD, B], BF16, tag="yst")
                last = bi == nblk - 1
                for d in range(KD):
                    w2t, dd = w2f_at(d)
                    pd = psum.tile([P, B], F32, tag="pd")
                    for jp in range(JH // 2):
                        nc.tensor.matmul(
                            pd[:],
                            w2t[:, dd * JH + 2 * jp : dd * JH + 2 * jp + 2],
                            hb[:, 2 * jp : 2 * jp + 2],
                            start=(jp == 0),
                            stop=(jp == JH // 2 - 1),
                            perf_mode=DR,
                        )
                    nc.scalar.activation(
                        yst[:, d],
                        pd[:],
                        mybir.ActivationFunctionType.Identity,
                        bias=b2sb[:, d : d + 1],
                        scale=1.0 / S2,
                    )
                    if last and d % 2 == 1:
                        # drain the finished pair right away so the
                        # end-of-kernel barrier waits on 2 planes, not 8
                        nc.sync.dma_start(
                            ypk[:, KD * off + (d - 1) * B : KD * off + (d + 1) * B],
                            yst[:, d - 1 : d + 1],
                        )
                if not last:
                    nc.sync.dma_start(ypk[:, KD * off : KD * (off + B)], yst[:])
                bi += 1

            # ── bf16 phase ───────────────────────────────────────────────
            yoff = cf
            for bj, (off, B) in enumerate(blocks_b):
                if bj == 0 and not havef:
                    xa, xc = xb0
                else:
                    xa, xc = load_block(off, B, nc.sync, xpk, BF16, "xb_")
                hb = hpool.tile([P, JH, B], BF16, tag="hb")
                for j in range(JH):
                    w1t, jj = w1b_at(j)
                    ph = psum.tile([P, B], F32, tag="ph")
                    for k in range(KD):
                        nc.tensor.matmul(
                            ph[:],
                            w1t[:, jj * KD + k],
                            xa[:, k] if k < KH else xc[:, k - KH],
                            start=(k == 0),
                            stop=(k == KD - 1),
                        )
                    nc.scalar.activation(
                        hb[:, j],
                        ph[:],
                        mybir.ActivationFunctionType.Gelu,
                        bias=b1sb[:, j : j + 1],
                    )
                    if bi == 0 and j < 4:
                        warm(4)
                yst = ypool.tile([P, KD, B], BF16, tag="yst")
                last = bi == nblk - 1
                oo = yoff + off
                for d in range(KD):
                    w2t, dd = w2b_at(d)
                    pd = psum.tile([P, B], F32, tag="pd")
                    for j in range(JH):
                        nc.tensor.matmul(
                            pd[:],
                            w2t[:, dd * JH + j],
                            hb[:, j],
                            start=(j == 0),
                            stop=(j == JH - 1),
                        )
                    nc.scalar.activation(
                        yst[:, d],
                        pd[:],
                        mybir.ActivationFunctionType.Identity,
                        bias=b2sb[:, d : d + 1],
                    )
                    if last and d % 2 == 1:
                        nc.sync.dma_start(
                            ypk[:, KD * oo + (d - 1) * B : KD * oo + (d + 1) * B],
                            yst[:, d - 1 : d + 1],
                        )
                if not last:
                    nc.sync.dma_start(ypk[:, KD * oo : KD * (oo + B)], yst[:])
                bi += 1
    _split_excess_waits(nc)
    return nc


_NC_CACHE = {}


def _routing(x, Wg, bg):
    """Gating computed the same way (and on the same platform: CPU jax) as the
    reference, so the top-2 choice is bit-identical even for near-tie logits."""
    import jax
    import jax.numpy as jnp

    cpu = jax.local_devices(backend="cpu")[0]
    with jax.default_device(cpu):
        logits = jnp.asarray(x) @ jnp.asarray(Wg) + jnp.asarray(bg)
        probs = jax.nn.softmax(logits, axis=-1)
        topk_p, topk_i = jax.lax.top_k(probs, TOP_K)
        topk_p = topk_p / topk_p.sum(axis=-1, keepdims=True)
    return np.asarray(topk_i), np.asarray(topk_p)


def _pack_x(xg, C, blocks, dt):
    """xg (C, D) -> (P, KD*C): per block, k-major then token-major, so each
    xa/xc DMA reads one contiguous run per partition."""
    x3 = np.asarray(xg, dtype=dt).reshape(C, KD, P)
    parts = [
        np.transpose(x3[off : off + B], (2, 1, 0)).reshape(P, KD * B)
        for off, B in blocks
    ]
    return np.ascontiguousarray(np.concatenate(parts, axis=1))


def _unpack_y(ypk, C, blocks):
    """(P, KD*C) bf16 -> (C, D) fp32, inverse of the yst tile layout."""
    y = np.empty((C, D), np.float32)
    for off, B in blocks:
        blk = ypk[:, KD * off : KD * (off + B)].reshape(P, KD, B)
        y[off : off + B] = np.transpose(blk, (2, 1, 0)).reshape(B, D)
    return y


def _pack_w1(w, dt):
    # w1pk[p, j, k, q] = w[k*P+p, j*P+q]
    return np.ascontiguousarray(
        np.transpose(np.asarray(w, dtype=dt).reshape(KD, P, JH, P), (1, 2, 0, 3)
                     ).reshape(P, JH * KD * P))


def _pack_w2(w, dt):
    # w2pk[p, d, j, q] = w[j*P+p, d*P+q]
    return np.ascontiguousarray(
        np.transpose(np.asarray(w, dtype=dt).reshape(JH, P, KD, P), (1, 2, 0, 3)
                     ).reshape(P, KD * JH * P))


def kernel(x, Wg, bg, W1, b1, W2, b2):
    global LAST_RESULTS
    import ml_dtypes

    bf16 = ml_dtypes.bfloat16
    fp8 = ml_dtypes.float8_e4m3
    x = np.ascontiguousarray(np.asarray(x, dtype=np.float32))
    Wg = np.asarray(Wg, dtype=np.float32)
    bg = np.asarray(bg, dtype=np.float32)
    W1 = np.asarray(W1, dtype=np.float32)
    b1 = np.asarray(b1, dtype=np.float32)
    W2 = np.asarray(W2, dtype=np.float32)
    b2 = np.asarray(b2, dtype=np.float32)

    topk_i, topk_p = _routing(x, Wg, bg)

    # Per expert: pairs sorted by routing weight ascending. Lowest-p pairs
    # beyond capacity spill to the exact host path; of the on-device pairs
    # the top C_B by p go to the bf16 class, the rest to the fp8 class.
    idx_list, p_list = [], []
    overflow = []
    for e in range(E):
        m0 = topk_i[:, 0] == e
        m1 = topk_i[:, 1] == e
        idx = np.nonzero(m0 | m1)[0]
        p = np.where(m0[idx], topk_p[idx, 0], topk_p[idx, 1]).astype(np.float32)
        order = np.argsort(p, kind="stable")
        idx, p = idx[order], p[order]
        n = len(idx)
        if n > CAP:
            overflow.append((e, idx[: n - CAP], p[: n - CAP]))
            idx, p = idx[n - CAP :], p[n - CAP :]
        idx_list.append(idx)
        p_list.append(p)

    blocks_f = _plan_blocks_f(C_F) if C_F else []
    blocks_b = _plan_blocks(C_B) if C_B else []

    key = (C_B, C_F)
    if key not in _NC_CACHE:
        _NC_CACHE[key] = build_nc(C_B, C_F)
    nc = _NC_CACHE[key]

    in_maps = []
    nf_list, nb_list = [], []
    for e in range(E):
        idx = idx_list[e]
        n = len(idx)
        nb = min(C_B, n)            # top-p pairs -> bf16 class
        nf = min(C_F, n - nb)       # rest -> fp8 class
        nf_list.append(nf)
        nb_list.append(nb)
        im = {
            "b1v": np.ascontiguousarray(b1[e].reshape(JH, P).T),
            "b2v": np.ascontiguousarray(b2[e].reshape(KD, P).T),
        }
        if C_F:
            xgf = np.zeros((C_F, D), np.float32)
            xgf[:nf] = x[idx[:nf]]
            im["xpkf"] = _pack_x(xgf, C_F, blocks_f, fp8)
            im["w1pkf"] = _pack_w1(W1[e] * S1, fp8)
            im["w2pkf"] = _pack_w2(W2[e] * S2, fp8)
        if C_B:
            xgb = np.zeros((C_B, D), np.float32)
            xgb[:nb] = x[idx[nf : nf + nb]]
            im["xpk"] = _pack_x(xgb, C_B, blocks_b, bf16)
            im["w1pk"] = _pack_w1(W1[e], bf16)
            im["w2pk"] = _pack_w2(W2[e], bf16)
        in_maps.append(im)

    res = run_bass_kernel_spmd(
        nc, in_maps, core_ids=list(range(E)), trace=TRACE, trace_cores=TRACE_CORES
    )
    LAST_RESULTS = res

    out = x.copy()
    for e in range(E):
        idx, p = idx_list[e], p_list[e]
        nf, nb = nf_list[e], nb_list[e]
        ype = np.asarray(res.results[e]["ypk"], np.float32)
        yf = _unpack_y(ype[:, : KD * C_F], C_F, blocks_f) if C_F else None
        yb = _unpack_y(ype[:, KD * C_F :], C_B, blocks_b) if C_B else None
        if nf:
            out[idx[:nf]] += yf[:nf] * p[:nf, None]
        if nb:
            out[idx[nf : nf + nb]] += yb[:nb] * p[nf : nf + nb, None]
    if overflow:
        import jax
        import jax.numpy as jnp

        cpu = jax.local_devices(backend="cpu")[0]
        with jax.default_device(cpu):
            for e, didx, dp in overflow:
                h = jax.nn.gelu(
                    jnp.asarray(x[didx]) @ jnp.asarray(W1[e]) + b1[e],
                    approximate=False,
                )
                ye = np.asarray(h @ jnp.asarray(W2[e]) + b2[e])
                out[didx] += ye * dp[:, None]
    return out
